# revision 14
# baseline (speedup 1.0000x reference)
"""Trainium2 Bass kernel for nn_MultiHeadAttention_78134045049371.

Strategy (8 NeuronCores, batch x head hybrid sharding):
  - Cores are split into 2 groups of 4 (group = batch). Core (b, g) owns
    batch b and heads [4g, 4g+4) — together a group covers all 16 heads
    of its batch, so the fc AllGather only spans 4 ranks and the two
    groups' collectives run in parallel on disjoint dies. Per-core gather
    wire traffic drops ~2.4x vs head-only sharding, and input DMA halves
    (each core loads only its batch).
  - Host feeds per-batch q/k/v TRANSPOSED and pre-tiled ([128, nt, c, 512]
    fp16) plus per-core weight slices, so every matmul contracts over the
    partition axis and every DMA partition line is one contiguous 8KB run.
  - The 4 heads per core are processed as 2 head-pairs (hp); each
    (q-block, hp) attention unit is the classic transposed-scores loop:
    S^T [keys, q] with the two heads in PE row groups (0,0)/(64,0), exp on
    [128, 2*QB] PSUM spans, and the "ones column" trick so O_aug =
    [V|1]^T @ exp(S^T) accumulates unnormalized output + exp-sum rows.
  - Contributions ([130, QB] per hp: 2 heads x (64 O rows + sumexp)) are
    AllGathered per q-block across the 4 group ranks -> [8*130, QB]
    "super-rank" (rank, hp) layout, identical to an 8-rank head-split
    gather, so fc-side softmax normalization (wide reciprocal + DRAM
    broadcast bounce) applies unchanged.
  - Final gated projection: core (b, g) computes its 256 output columns
    (2 out-tiles of 128) of sigmoid(O@Wg^T) * tanh(O@Wfc^T) for its
    batch's 2048 rows. Output stays transposed; host reassembles.
  - Queue assignment: input streams + fc input DMAs on sync, contribution
    and result DMAs + collective triggers on gpsimd, broadcast reads on
    gpsimd. All matmuls fp16 (fp32 PSUM accumulation); exp/tanh fp32 ACT.
"""

import sys

for _p in ("/opt/trn_rl_repo", "/root/.axon_site/_ro/trn_rl_repo"):
    if _p not in sys.path:
        sys.path.append(_p)

import numpy as np

import concourse.bass as bass
import concourse.mybir as mybir
import concourse.tile as tile
from concourse import bass_utils
from concourse.vector_clock import ScopedClock

# Problem shape (fixed by the reference)
B, L, D = 2, 2048, 1024
H, DK, DV = 16, 64, 64
NC = 8  # cores
NG = 4  # cores per group (= ranks per AllGather)
HC = H // NG  # heads per core = 4
HL = 2  # heads per head-pair (attention unit)
NHP = HC // HL  # head-pairs per core = 2
TEMP = float(np.sqrt(DK))  # 8.0

NQB = 4  # q-blocks per core (one batch: L rows)
QB = L // NQB  # 512 columns per q-block
KT = 128  # key tile (partition dim of S^T)
NKT = L // KT  # 16 key tiles
DCH = D // 128  # 8 contraction chunks of 128
SR = NG * NHP  # "super-ranks" in the gathered buffer = 8
CR = HL * (DV + 1)  # contribution rows per head-pair = 130

F16 = mybir.dt.float16
F32 = mybir.dt.float32

MAX_WAITS = 1  # this walrus build encodes at most 1 sem-wait per instruction


def _split_excess_waits(nc):
    """Move excess sem-waits onto NOPs inserted just before the owning
    instruction on the same engine (engine queues are FIFO, so semantics
    are preserved). The walrus build here rejects >1 wait per instruction."""
    for f in nc.m.functions:
        for bb in f.blocks:
            out = []
            changed = False
            for inst in bb.instructions:
                si = inst.sync_info
                waits = list(si.on_wait) if si and si.on_wait else []
                if len(waits) > MAX_WAITS:
                    changed = True
                    k = 0
                    while len(waits) > MAX_WAITS:
                        chunk, waits = waits[:MAX_WAITS], waits[MAX_WAITS:]
                        nop = mybir.InstNoOp(
                            name=f"{inst.name}-wsplit-{k}", ins=[], outs=[]
                        )
                        nop.engine = inst.engine
                        nop.sync_info = mybir.SyncInfo(on_wait=chunk, on_update=[])
                        nc.register_instruction(nop, overwrite=True)
                        out.append(nop)
                        k += 1
                    si.on_wait = waits
                    inst.sync_info = si
                out.append(inst)
            if changed:
                bb.instructions = out


class _TileContext(tile.TileContext):
    """TileContext whose final drain carries its waits on separate NOPs."""

    def _drain_and_barrier(self, tick_clock, wait_clock):
        nc = self.nc
        collector = nc.sync.nop(nofuse=True)
        wait_clock.add_sem_waits(
            collector.ins, ScopedClock({None: tick_clock.global_clock})
        )
        nc.sync.drain()
        nc.all_engine_barrier()
        popped = nc._tile_sem_poison_stack.pop()
        assert popped is self._sem_poison
        nc.clear_and_free_semaphores(list(self.sems.allocated().values()))
        nc.all_engine_barrier()

    def __exit__(self, exc_type, exc_value, traceback):
        super().__exit__(exc_type, exc_value, traceback)
        if exc_type is None:
            _split_excess_waits(self.nc)


def build_kernel():
    nc = bass.Bass(target_bir_lowering=False)

    # Inputs (per core): pre-tiled transposed activations for THIS CORE'S
    # BATCH and per-core weight slices, all fp16. Layout [128, nt, c, n]:
    # partition p holds contraction-row c*128+p, column block nt.
    qT = nc.dram_tensor("qT", [128, NQB, DCH, 512], F16, kind="ExternalInput")
    kT = nc.dram_tensor("kT", [128, NQB, DCH, 512], F16, kind="ExternalInput")
    vT = nc.dram_tensor("vT", [128, NQB, DCH, 512], F16, kind="ExternalInput")
    # [128, c, 256]: columns = this core's 4 heads x 64 (q pre-scaled 1/8)
    wqT = nc.dram_tensor("wqT", [128, DCH, HC * DK], F16, kind="ExternalInput")
    wkT = nc.dram_tensor("wkT", [128, DCH, HC * DK], F16, kind="ExternalInput")
    wvT = nc.dram_tensor("wvT", [128, DCH, HC * DV], F16, kind="ExternalInput")
    # [128, c, 256]: this core's 256 output columns of Wfc/Wg, contraction
    # row c*128+p (= super-rank c's head block of H*DV)
    wfcT = nc.dram_tensor("wfcT", [128, DCH, 2 * 128], F16, kind="ExternalInput")
    wgT = nc.dram_tensor("wgT", [128, DCH, 2 * 128], F16, kind="ExternalInput")

    # Output: this core's 256 output columns for its batch's L rows,
    # stored transposed ([dout, row]); the host does the final transpose.
    out = nc.dram_tensor("out", [2 * 128, L], F32, kind="ExternalOutput")

    # AllGather buffers: per q-block contribution [2*130, QB] (per head:
    # 64 unnormalized O^T rows + exp-sum row, x2 head-pairs) -> gathered
    # [NG*260, QB] = [SR*130, QB] with super-rank (rank, hp) on dim 0.
    ag_in = nc.dram_tensor("ag_in", [NQB, NHP * CR, QB], F16)
    ag_out = nc.dram_tensor("ag_out", [NQB, NG * NHP * CR, QB], F16)
    # 1/sumexp rows, [16, QB] per q-block (SR x HL heads), bounced via
    # DRAM so they can be broadcast-read across partitions (SBUF sources
    # cannot have partition-step-0 APs, DRAM sources can)
    recD = nc.dram_tensor("recD", [NQB, SR * HL, QB], F16)
    # tiny warm-up collective: absorbs the one-time ncfw/collective setup
    # (~20-40us observed on the first AllGather) off the critical path
    warm_in = nc.dram_tensor("warm_in", [1, 64], F16)
    warm_out = nc.dram_tensor("warm_out", [NG, 64], F16)

    GROUPS = [[0, 1, 2, 3], [4, 5, 6, 7]]

    with _TileContext(nc) as tc:
        with (
            tc.tile_pool(name="persist", bufs=1) as persist,
            tc.tile_pool(name="astream", bufs=4) as astream,
            tc.tile_pool(name="exps", bufs=10) as exps,
            tc.tile_pool(name="small", bufs=3) as small,
            tc.tile_pool(name="fcin", bufs=2) as fcin,
            tc.tile_pool(name="pp_o", bufs=2, space="PSUM") as pp_o,
            tc.tile_pool(name="pp_fc", bufs=2, space="PSUM") as pp_fc,
            tc.tile_pool(name="pp_s", bufs=2, space="PSUM") as pp_s,
        ):
            # ---- resident tiles (indexed by head-pair hp) ----
            qhTs = [
                persist.tile([HL * DK, QB], F16, name=f"qhT{i}")
                for i in range(NQB * NHP)  # index qb*NHP + hp
            ]
            khTs = [
                persist.tile([HL * DK, L], F16, name=f"khT{i}") for i in range(NHP)
            ]
            # vh augmented with a ones column per head: [head][0:64]=vh, [64]=1
            vhs = [
                persist.tile([128, L // 128, HL * (DV + 1)], F16, name=f"vh{i}")
                for i in range(NHP)
            ]
            wfc_sb = persist.tile([128, DCH, 2 * 128], F16)
            wg_sb = persist.tile([128, DCH, 2 * 128], F16)

            # ones columns of vh (written once; matmul copies never touch them)
            for vh in vhs:
                nc.vector.memset(vh[:, :, DV : DV + 1], 1.0)
                nc.vector.memset(vh[:, :, DV + 1 + DV :], 1.0)

            # ---- projection weights ----
            wq_sb = persist.tile([128, DCH, HC * DK], F16)
            wk_sb = persist.tile([128, DCH, HC * DK], F16)
            wv_sb = persist.tile([128, DCH, HC * DV], F16)
            nc.sync.dma_start(out=wk_sb[:], in_=wkT[:])
            nc.sync.dma_start(out=wv_sb[:], in_=wvT[:])
            nc.sync.dma_start(out=wq_sb[:], in_=wqT[:])
            nc.sync.dma_start(out=wfc_sb[:], in_=wfcT[:])
            nc.sync.dma_start(out=wg_sb[:], in_=wgT[:])

            # ---- projections (input DMAs issued upfront on the sync
            # queue so later output traffic can never starve them) ----
            def load_x(src, nt, tag, bufs):
                xt = astream.tile(
                    [128, DCH, 512], F16, tag=tag, bufs=bufs, name="xt"
                )
                nc.sync.dma_start(out=xt[:], in_=src[:, nt])
                return xt

            def proj_kq(xt, wsb, dsts):
                # dsts[hp] [128, 512] = sum_c w[c, hp].T @ xT[c]
                for hp in range(NHP):
                    ps = pp_fc.tile([128, 512], F32, tag="fcpsum", name="psq")
                    for c in range(DCH):
                        nc.tensor.matmul(
                            ps[:],
                            lhsT=wsb[:, c, hp * 128 : (hp + 1) * 128],
                            rhs=xt[:, c, :],
                            start=(c == 0),
                            stop=(c == DCH - 1),
                        )
                    nc.vector.tensor_copy(out=dsts[hp][:], in_=ps[:])

            def proj_v(vt, nt):
                for sub in range(4):
                    loc = nt * 4 + sub
                    ps = pp_fc.tile([128, 512], F32, tag="fcpsum", name="psv")
                    for hp in range(NHP):
                        for c in range(DCH):
                            nc.tensor.matmul(
                                ps[:, hp * 128 : (hp + 1) * 128],
                                lhsT=vt[:, c, bass.ts(sub, 128)],
                                rhs=wv_sb[:, c, hp * 128 : (hp + 1) * 128],
                                start=(c == 0),
                                stop=(c == DCH - 1),
                            )
                    for hp in range(NHP):
                        for h in range(HL):
                            nc.vector.tensor_copy(
                                out=vhs[hp][
                                    :, loc, h * (DV + 1) : h * (DV + 1) + DV
                                ],
                                in_=ps[:, (hp * HL + h) * DV : (hp * HL + h + 1) * DV],
                            )

            # ---- attention per (q-block, head-pair) ----
            # S matmuls for the two heads sit at PE row groups (0,0)/(64,0)
            # and execute concurrently. exp runs on [128, 2*QB] PSUM spans.
            # Unnormalized O rows + exp-sum rows ship into the per-q-block
            # AllGather; normalization happens post-gather on the fc side.
            def attention(qb, hp):
                opsums = [
                    pp_o.tile([DV + 1, QB], F32, tag="opsum", name=f"ops{h}")
                    for h in range(HL)
                ]
                for kt in range(NKT):
                    sps = pp_s.tile([KT, HL * QB], F32, tag="spsum")
                    for h in range(HL):
                        hp_ = h * DK
                        nc.tensor.matmul(
                            sps[:, h * QB : (h + 1) * QB],
                            lhsT=khTs[hp][hp_ : hp_ + DK, kt * KT : (kt + 1) * KT],
                            rhs=qhTs[qb * NHP + hp][hp_ : hp_ + DK, :],
                            start=True,
                            stop=True,
                        )
                    et = exps.tile([KT, HL * QB], F16, tag="expst")
                    nc.scalar.activation(
                        out=et[:],
                        in_=sps[:],
                        func=mybir.ActivationFunctionType.Exp,
                    )
                    for h in range(HL):
                        nc.tensor.matmul(
                            opsums[h][:],
                            lhsT=vhs[hp][:, kt, h * (DV + 1) : (h + 1) * (DV + 1)],
                            rhs=et[:, h * QB : (h + 1) * QB],
                            start=(kt == 0),
                            stop=(kt == NKT - 1),
                        )
                for h in range(HL):
                    ctile = small.tile([DV + 1, QB], F16, tag="contrib", name="ct")
                    nc.vector.tensor_copy(out=ctile[:], in_=opsums[h][:])
                    nc.sync.dma_start(
                        out=ag_in[
                            qb,
                            hp * CR + h * (DV + 1) : hp * CR + (h + 1) * (DV + 1),
                        ],
                        in_=ctile[:],
                    )

            def allgather(qb):
                nc.gpsimd.collective_compute(
                    "AllGather",
                    mybir.AluOpType.bypass,
                    replica_groups=GROUPS,
                    ins=[ag_in[qb]],
                    outs=[ag_out[qb]],
                )

            # ---- gated output projection for this core's 256 columns ----
            def fc_block(qb):
                # super-rank s = (rank, hp) carries heads 2s..2s+1; gathered
                # block s = contraction chunk s of Wfc/Wg
                ago = ag_out[qb].rearrange("(s h x) q -> s h x q", h=HL, x=DV + 1)
                # reciprocal of all 16 exp-sum rows at once, reshaped to
                # [128, 64] so the reciprocal runs on 128 lanes, then bounced
                # to DRAM for partition-broadcast reads.
                sums_sb = small.tile([128, (SR * HL * QB) // 128], F16, tag="sums")
                nc.sync.dma_start(
                    out=sums_sb[:],
                    in_=ago[:, :, DV, :].rearrange(
                        "s h (a f) -> s h a f", f=(SR * HL * QB) // 128
                    ),
                )
                rec_sb = small.tile([128, (SR * HL * QB) // 128], F16, tag="recs")
                with nc.allow_low_precision(reason="softmax normalizer in fp16"):
                    nc.vector.reciprocal(out=rec_sb[:], in_=sums_sb[:])
                nc.sync.dma_start(
                    out=recD[qb].rearrange(
                        "s (a f) -> (s a) f", f=(SR * HL * QB) // 128
                    ),
                    in_=rec_sb[:],
                )
                # all 8 chunks' O rows + 1/sumexp broadcasts
                ot_all = fcin.tile([128, DCH, QB], F16, tag="fcin", name="ot_all")
                rs_all = fcin.tile([128, DCH, QB], F16, tag="fcrs", name="rs_all")
                for h in range(HL):
                    nc.sync.dma_start(
                        out=ot_all[h * DV : (h + 1) * DV],
                        in_=ago[:, h, :DV, :].rearrange("s x q -> x s q"),
                    )
                    nc.sync.dma_start(
                        out=rs_all[h * DV : (h + 1) * DV],
                        in_=recD[qb]
                        .rearrange("(c h) q -> h c q", h=HL)[h][None, :, :]
                        .to_broadcast([DV, DCH, QB]),
                    )
                nc.vector.tensor_mul(out=ot_all[:], in0=ot_all[:], in1=rs_all[:])
                for t in range(2):  # two 128-column output tiles
                    fps = pp_fc.tile([128, QB], F32, tag="fcpsum", name="fps")
                    gps = pp_fc.tile([128, QB], F32, tag="fcpsum", name="gps")
                    for c in range(DCH):
                        nc.tensor.matmul(
                            fps[:],
                            lhsT=wfc_sb[:, c, t * 128 : (t + 1) * 128],
                            rhs=ot_all[:, c, :],
                            start=(c == 0),
                            stop=(c == DCH - 1),
                        )
                    for c in range(DCH):
                        nc.tensor.matmul(
                            gps[:],
                            lhsT=wg_sb[:, c, t * 128 : (t + 1) * 128],
                            rhs=ot_all[:, c, :],
                            start=(c == 0),
                            stop=(c == DCH - 1),
                        )
                    # sigmoid(g) = 0.5*tanh(g/2) + 0.5 — stays on the
                    # exp/tanh table set (no ~2.7us table reloads)
                    tanh_t = small.tile([128, QB], F32, tag="tanh")
                    sig_t = small.tile([128, QB], F32, tag="sig")
                    nc.scalar.activation(
                        out=tanh_t[:], in_=fps[:],
                        func=mybir.ActivationFunctionType.Tanh,
                    )
                    nc.scalar.activation(
                        out=sig_t[:], in_=gps[:],
                        func=mybir.ActivationFunctionType.Tanh, scale=0.5,
                    )
                    nc.vector.tensor_scalar(
                        out=sig_t[:],
                        in0=sig_t[:],
                        scalar1=0.5,
                        scalar2=0.5,
                        op0=mybir.AluOpType.mult,
                        op1=mybir.AluOpType.add,
                    )
                    res = small.tile([128, QB], F32, tag="res")
                    nc.vector.tensor_mul(out=res[:], in0=sig_t[:], in1=tanh_t[:])
                    nc.sync.dma_start(
                        out=out[t * 128 : (t + 1) * 128, bass.ts(qb, QB)],
                        in_=res[:],
                    )

            # ---- emission order: ALL input DMAs first (k, v, then q; the
            # q loads reuse the k ring slots and flow as k-proj retires),
            # then keys/values projected, q-blocks just-in-time, AllGather
            # per q-block after both head-pairs, fc blocks one AllGather
            # behind. The gpsimd queue carries ONLY collective triggers
            # (a trigger blocks its queue until the collective completes,
            # so nothing else may sit behind one). ----
            wtile = small.tile([1, 64], F16, tag="warm", name="wtile")
            nc.vector.memset(wtile[:], 0.0)
            nc.sync.dma_start(out=warm_in[:], in_=wtile[:])
            nc.gpsimd.collective_compute(
                "AllGather",
                mybir.AluOpType.bypass,
                replica_groups=GROUPS,
                ins=[warm_in[:]],
                outs=[warm_out[:]],
            )
            kxs = [load_x(kT, nt, "xproj", 6) for nt in range(NQB)]
            vxs = [load_x(vT, nt, "vproj", 4) for nt in range(NQB)]
            qxs = [load_x(qT, nt, "xproj", 6) for nt in range(NQB)]
            for nt in range(NQB):  # keys, both head-pairs
                proj_kq(kxs[nt], wk_sb, [khTs[0][:, bass.ts(nt, 512)],
                                         khTs[1][:, bass.ts(nt, 512)]])
            for nt in range(NQB):  # values
                proj_v(vxs[nt], nt)
            proj_kq(qxs[0], wq_sb, [qhTs[0], qhTs[1]])
            attention(0, 0)
            proj_kq(qxs[1], wq_sb, [qhTs[2], qhTs[3]])
            attention(0, 1)
            allgather(0)
            attention(1, 0)
            proj_kq(qxs[2], wq_sb, [qhTs[4], qhTs[5]])
            attention(1, 1)
            allgather(1)
            fc_block(0)
            attention(2, 0)
            proj_kq(qxs[3], wq_sb, [qhTs[6], qhTs[7]])
            attention(2, 1)
            allgather(2)
            fc_block(1)
            attention(3, 0)
            fc_block(2)
            attention(3, 1)
            allgather(3)
            fc_block(3)

    return nc


_NC_CACHE = None


def _get_nc():
    global _NC_CACHE
    if _NC_CACHE is None:
        _NC_CACHE = build_kernel()
    return _NC_CACHE


def prepare_inputs(q, k, v, Wq, bq, Wk, bk, Wv, bv, Wfc, bfc, Wg, bg):
    """Host-side layout prep: transpose + fp16 cast + per-core slices.

    Core c = (batch c//4, head-group c%4). Activations are pre-tiled to
    [128, nt, c, n] so each DMA partition line is one contiguous 8KB run.
    Biases are structurally zero in this problem and are folded out.
    """

    def tile_act(xb):
        # [L, D] -> [D, L] -> [c, p, nt, n] -> [p, nt, c, n]
        xT = np.ascontiguousarray(np.asarray(xb).reshape(L, D).T, np.float16)
        return np.ascontiguousarray(
            xT.reshape(DCH, 128, NQB, 512).transpose(1, 2, 0, 3)
        )

    def tile_w(wT):
        # [D, M] -> [c, p, M] -> [p, c, M]
        return np.ascontiguousarray(
            wT.reshape(DCH, 128, wT.shape[1]).transpose(1, 0, 2)
        )

    acts = [[tile_act(x[b]) for b in range(B)] for x in (q, k, v)]
    WqT = (np.asarray(Wq, np.float32) / TEMP).T.astype(np.float16)  # [D, H*DK]
    WkT = np.asarray(Wk, np.float32).T.astype(np.float16)
    WvT = np.asarray(Wv, np.float32).T.astype(np.float16)
    WfcT = np.asarray(Wfc, np.float32).T.astype(np.float16)  # [H*DV, D]
    WgT = np.asarray(Wg, np.float32).T.astype(np.float16)

    in_maps = []
    for c in range(NC):
        b, g = c // NG, c % NG
        hs = g * HC * DK  # 256-wide head slice
        in_maps.append(
            {
                "qT": acts[0][b],
                "kT": acts[1][b],
                "vT": acts[2][b],
                "wqT": tile_w(WqT[:, hs : hs + HC * DK]),
                "wkT": tile_w(WkT[:, hs : hs + HC * DK]),
                "wvT": tile_w(WvT[:, hs : hs + HC * DV]),
                "wfcT": tile_w(WfcT[:, g * 256 : (g + 1) * 256]),
                "wgT": tile_w(WgT[:, g * 256 : (g + 1) * 256]),
            }
        )
    return in_maps


def assemble_output(results):
    # core (b, g) produced output columns [g*256, (g+1)*256) of batch b,
    # transposed [256, L]
    batches = []
    for b in range(B):
        cols = [results[b * NG + g]["out"] for g in range(NG)]
        full = np.concatenate(cols, axis=0)  # [D, L]
        batches.append(np.ascontiguousarray(full.T))  # [L, D]
    return np.stack(batches, axis=0)


def kernel(**inputs):
    nc = _get_nc()
    in_maps = prepare_inputs(**{k: np.asarray(v) for k, v in inputs.items()})
    res = bass_utils.run_bass_kernel_spmd(nc, in_maps, core_ids=list(range(NC)))
    return assemble_output(res.results)


if __name__ == "__main__":
    nc = build_kernel()
    print("kernel built OK")


# revision 17
# speedup vs baseline: 1.3349x; 1.3349x over previous
"""Trainium2 Bass kernel for nn_MultiHeadAttention_78134045049371.

Strategy (8 NeuronCores, batch x head hybrid sharding):
  - Cores are split into 2 groups of 4 (group = batch). Core (b, g) owns
    batch b and heads [4g, 4g+4) — together a group covers all 16 heads
    of its batch, so the fc AllGather only spans 4 ranks and the two
    groups' collectives run in parallel on disjoint dies. Per-core gather
    wire traffic drops ~2.4x vs head-only sharding, and input DMA halves
    (each core loads only its batch).
  - Host feeds per-batch q/k/v TRANSPOSED and pre-tiled ([128, nt, c, 512]
    fp16) plus per-core weight slices, so every matmul contracts over the
    partition axis and every DMA partition line is one contiguous 8KB run.
  - The 4 heads per core are processed as 2 head-pairs (hp); each
    (q-block, hp) attention unit is the classic transposed-scores loop:
    S^T [keys, q] with the two heads in PE row groups (0,0)/(64,0), exp on
    [128, 2*QB] PSUM spans, and the "ones column" trick so O_aug =
    [V|1]^T @ exp(S^T) accumulates unnormalized output + exp-sum rows.
  - Contributions ([130, QB] per hp: 2 heads x (64 O rows + sumexp)) are
    AllGathered per q-block across the 4 group ranks -> [8*130, QB]
    "super-rank" (rank, hp) layout, identical to an 8-rank head-split
    gather, so fc-side softmax normalization (wide reciprocal + DRAM
    broadcast bounce) applies unchanged.
  - Final gated projection: core (b, g) computes its 256 output columns
    (2 out-tiles of 128) of sigmoid(O@Wg^T) * tanh(O@Wfc^T) for its
    batch's 2048 rows. Output stays transposed; host reassembles.
  - Queue assignment: input streams + fc input DMAs on sync, contribution
    and result DMAs + collective triggers on gpsimd, broadcast reads on
    gpsimd. All matmuls fp16 (fp32 PSUM accumulation); exp/tanh fp32 ACT.
"""

import sys

for _p in ("/opt/trn_rl_repo", "/root/.axon_site/_ro/trn_rl_repo"):
    if _p not in sys.path:
        sys.path.append(_p)

import numpy as np

import concourse.bass as bass
import concourse.mybir as mybir
import concourse.tile as tile
from concourse import bass_utils
from concourse.vector_clock import ScopedClock

# Problem shape (fixed by the reference)
B, L, D = 2, 2048, 1024
H, DK, DV = 16, 64, 64
NC = 8  # cores
NG = 4  # cores per group (= ranks per AllGather)
HC = H // NG  # heads per core = 4
HL = 2  # heads per head-pair (attention unit)
NHP = HC // HL  # head-pairs per core = 2
TEMP = float(np.sqrt(DK))  # 8.0

NQB = 4  # q-blocks per core (one batch: L rows)
QB = L // NQB  # 512 columns per q-block
KT = 128  # key tile (partition dim of S^T)
NKT = L // KT  # 16 key tiles
DCH = D // 128  # 8 contraction chunks of 128
SR = NG * NHP  # "super-ranks" in the gathered buffer = 8
CR = HL * (DV + 1)  # contribution rows per head-pair = 130

F16 = mybir.dt.float16
F32 = mybir.dt.float32

MAX_WAITS = 1  # this walrus build encodes at most 1 sem-wait per instruction


def _split_excess_waits(nc):
    """Move excess sem-waits onto NOPs inserted just before the owning
    instruction on the same engine (engine queues are FIFO, so semantics
    are preserved). The walrus build here rejects >1 wait per instruction."""
    for f in nc.m.functions:
        for bb in f.blocks:
            out = []
            changed = False
            for inst in bb.instructions:
                si = inst.sync_info
                waits = list(si.on_wait) if si and si.on_wait else []
                if len(waits) > MAX_WAITS:
                    changed = True
                    k = 0
                    while len(waits) > MAX_WAITS:
                        chunk, waits = waits[:MAX_WAITS], waits[MAX_WAITS:]
                        nop = mybir.InstNoOp(
                            name=f"{inst.name}-wsplit-{k}", ins=[], outs=[]
                        )
                        nop.engine = inst.engine
                        nop.sync_info = mybir.SyncInfo(on_wait=chunk, on_update=[])
                        nc.register_instruction(nop, overwrite=True)
                        out.append(nop)
                        k += 1
                    si.on_wait = waits
                    inst.sync_info = si
                out.append(inst)
            if changed:
                bb.instructions = out


class _TileContext(tile.TileContext):
    """TileContext whose final drain carries its waits on separate NOPs."""

    def _drain_and_barrier(self, tick_clock, wait_clock):
        nc = self.nc
        collector = nc.sync.nop(nofuse=True)
        wait_clock.add_sem_waits(
            collector.ins, ScopedClock({None: tick_clock.global_clock})
        )
        nc.sync.drain()
        nc.all_engine_barrier()
        popped = nc._tile_sem_poison_stack.pop()
        assert popped is self._sem_poison
        nc.clear_and_free_semaphores(list(self.sems.allocated().values()))
        nc.all_engine_barrier()

    def __exit__(self, exc_type, exc_value, traceback):
        super().__exit__(exc_type, exc_value, traceback)
        if exc_type is None:
            _split_excess_waits(self.nc)


def build_kernel():
    nc = bass.Bass(target_bir_lowering=False)

    # Inputs (per core): pre-tiled transposed activations for THIS CORE'S
    # BATCH and per-core weight slices, all fp16. Layout [128, nt, c, n]:
    # partition p holds contraction-row c*128+p, column block nt.
    qT = nc.dram_tensor("qT", [128, NQB, DCH, 512], F16, kind="ExternalInput")
    kT = nc.dram_tensor("kT", [128, NQB, DCH, 512], F16, kind="ExternalInput")
    vT = nc.dram_tensor("vT", [128, NQB, DCH, 512], F16, kind="ExternalInput")
    # [128, c, 256]: columns = this core's 4 heads x 64 (q pre-scaled 1/8)
    wqT = nc.dram_tensor("wqT", [128, DCH, HC * DK], F16, kind="ExternalInput")
    wkT = nc.dram_tensor("wkT", [128, DCH, HC * DK], F16, kind="ExternalInput")
    wvT = nc.dram_tensor("wvT", [128, DCH, HC * DV], F16, kind="ExternalInput")
    # [128, c, 256]: this core's 256 output columns of Wfc/Wg, contraction
    # row c*128+p (= super-rank c's head block of H*DV)
    wfcT = nc.dram_tensor("wfcT", [128, DCH, 2 * 128], F16, kind="ExternalInput")
    wgT = nc.dram_tensor("wgT", [128, DCH, 2 * 128], F16, kind="ExternalInput")

    # Output: this core's 256 output columns for its batch's L rows,
    # stored transposed ([dout, row]); the host does the final transpose.
    out = nc.dram_tensor("out", [2 * 128, L], F32, kind="ExternalOutput")

    # AllGather buffers: per q-block contribution [2*130, QB] (per head:
    # 64 unnormalized O^T rows + exp-sum row, x2 head-pairs) -> gathered
    # [NG*260, QB] = [SR*130, QB] with super-rank (rank, hp) on dim 0.
    ag_in = nc.dram_tensor("ag_in", [NQB, NHP * CR, QB], F16)
    ag_out = nc.dram_tensor("ag_out", [NQB, NG * NHP * CR, QB], F16)
    # 1/sumexp rows, [16, QB] per q-block (SR x HL heads), bounced via
    # DRAM so they can be broadcast-read across partitions (SBUF sources
    # cannot have partition-step-0 APs, DRAM sources can)
    recD = nc.dram_tensor("recD", [NQB, SR * HL, QB], F16)

    GROUPS = [[0, 1, 2, 3], [4, 5, 6, 7]]

    with _TileContext(nc) as tc:
        with (
            tc.tile_pool(name="persist", bufs=1) as persist,
            tc.tile_pool(name="astream", bufs=4) as astream,
            tc.tile_pool(name="exps", bufs=10) as exps,
            tc.tile_pool(name="small", bufs=3) as small,
            tc.tile_pool(name="fcin", bufs=3) as fcin,
            tc.tile_pool(name="pp_o", bufs=2, space="PSUM") as pp_o,
            tc.tile_pool(name="pp_fc", bufs=2, space="PSUM") as pp_fc,
            tc.tile_pool(name="pp_s", bufs=2, space="PSUM") as pp_s,
        ):
            # ---- resident tiles (indexed by head-pair hp) ----
            qhTs = [
                persist.tile([HL * DK, QB], F16, name=f"qhT{i}")
                for i in range(NQB * NHP)  # index qb*NHP + hp
            ]
            khTs = [
                persist.tile([HL * DK, L], F16, name=f"khT{i}") for i in range(NHP)
            ]
            # vh augmented with a ones column per head: [head][0:64]=vh, [64]=1
            vhs = [
                persist.tile([128, L // 128, HL * (DV + 1)], F16, name=f"vh{i}")
                for i in range(NHP)
            ]
            wfc_sb = persist.tile([128, DCH, 2 * 128], F16)
            wg_sb = persist.tile([128, DCH, 2 * 128], F16)

            # ones columns of vh (written once; matmul copies never touch them)
            for vh in vhs:
                nc.vector.memset(vh[:, :, DV : DV + 1], 1.0)
                nc.vector.memset(vh[:, :, DV + 1 + DV :], 1.0)

            # ---- projection weights ----
            wq_sb = persist.tile([128, DCH, HC * DK], F16)
            wk_sb = persist.tile([128, DCH, HC * DK], F16)
            wv_sb = persist.tile([128, DCH, HC * DV], F16)
            nc.sync.dma_start(out=wk_sb[:], in_=wkT[:])

            # ---- projections ----
            def proj_kq(src, wsb, dsts, nt):
                # dsts[hp] [128, 512] = sum_c w[c, hp].T @ xT[c] for block nt
                xt = astream.tile([128, DCH, 512], F16, tag="xproj", name="xt")
                nc.sync.dma_start(out=xt[:], in_=src[:, nt])
                for hp in range(NHP):
                    ps = pp_fc.tile([128, 512], F32, tag="fcpsum", name="psq")
                    for c in range(DCH):
                        nc.tensor.matmul(
                            ps[:],
                            lhsT=wsb[:, c, hp * 128 : (hp + 1) * 128],
                            rhs=xt[:, c, :],
                            start=(c == 0),
                            stop=(c == DCH - 1),
                        )
                    nc.vector.tensor_copy(out=dsts[hp][:], in_=ps[:])

            def proj_v(nt):
                vt = astream.tile([128, DCH, 512], F16, tag="vproj", name="vt")
                nc.sync.dma_start(out=vt[:], in_=vT[:, nt])
                for sub in range(4):
                    loc = nt * 4 + sub
                    ps = pp_fc.tile([128, 512], F32, tag="fcpsum", name="psv")
                    for hp in range(NHP):
                        for c in range(DCH):
                            nc.tensor.matmul(
                                ps[:, hp * 128 : (hp + 1) * 128],
                                lhsT=vt[:, c, bass.ts(sub, 128)],
                                rhs=wv_sb[:, c, hp * 128 : (hp + 1) * 128],
                                start=(c == 0),
                                stop=(c == DCH - 1),
                            )
                    for hp in range(NHP):
                        for h in range(HL):
                            nc.vector.tensor_copy(
                                out=vhs[hp][
                                    :, loc, h * (DV + 1) : h * (DV + 1) + DV
                                ],
                                in_=ps[:, (hp * HL + h) * DV : (hp * HL + h + 1) * DV],
                            )

            # ---- attention per (q-block, head-pair) ----
            # S matmuls for the two heads sit at PE row groups (0,0)/(64,0)
            # and execute concurrently. exp runs on [128, 2*QB] PSUM spans.
            # Unnormalized O rows + exp-sum rows ship into the per-q-block
            # AllGather; normalization happens post-gather on the fc side.
            def attention(qb, hp):
                opsums = [
                    pp_o.tile([DV + 1, QB], F32, tag="opsum", name=f"ops{h}")
                    for h in range(HL)
                ]
                for kt in range(NKT):
                    sps = pp_s.tile([KT, HL * QB], F32, tag="spsum")
                    for h in range(HL):
                        hp_ = h * DK
                        nc.tensor.matmul(
                            sps[:, h * QB : (h + 1) * QB],
                            lhsT=khTs[hp][hp_ : hp_ + DK, kt * KT : (kt + 1) * KT],
                            rhs=qhTs[qb * NHP + hp][hp_ : hp_ + DK, :],
                            start=True,
                            stop=True,
                        )
                    et = exps.tile([KT, HL * QB], F16, tag="expst")
                    nc.scalar.activation(
                        out=et[:],
                        in_=sps[:],
                        func=mybir.ActivationFunctionType.Exp,
                    )
                    for h in range(HL):
                        nc.tensor.matmul(
                            opsums[h][:],
                            lhsT=vhs[hp][:, kt, h * (DV + 1) : (h + 1) * (DV + 1)],
                            rhs=et[:, h * QB : (h + 1) * QB],
                            start=(kt == 0),
                            stop=(kt == NKT - 1),
                        )
                for h in range(HL):
                    # deep ring: contribution DMAs sit on the gpsimd queue
                    # behind AllGather completion-waits; 6 bufs give the DVE
                    # casts ~3 attention-units of slack so the opsum ring
                    # never backs up into the O matmuls
                    ctile = small.tile(
                        [DV + 1, QB], F16, tag="contrib", bufs=6, name="ct"
                    )
                    nc.vector.tensor_copy(out=ctile[:], in_=opsums[h][:])
                    nc.gpsimd.dma_start(
                        out=ag_in[
                            qb,
                            hp * CR + h * (DV + 1) : hp * CR + (h + 1) * (DV + 1),
                        ],
                        in_=ctile[:],
                    )

            def allgather(qb):
                nc.gpsimd.collective_compute(
                    "AllGather",
                    mybir.AluOpType.bypass,
                    replica_groups=GROUPS,
                    ins=[ag_in[qb]],
                    outs=[ag_out[qb]],
                )

            # ---- gated output projection for this core's 256 columns ----
            def fc_block(qb):
                # super-rank s = (rank, hp) carries heads 2s..2s+1; gathered
                # block s = contraction chunk s of Wfc/Wg
                ago = ag_out[qb].rearrange("(s h x) q -> s h x q", h=HL, x=DV + 1)
                # reciprocal of all 16 exp-sum rows at once, reshaped to
                # [128, 64] so the reciprocal runs on 128 lanes, then bounced
                # to DRAM for partition-broadcast reads.
                sums_sb = small.tile([128, (SR * HL * QB) // 128], F16, tag="sums")
                nc.sync.dma_start(
                    out=sums_sb[:],
                    in_=ago[:, :, DV, :].rearrange(
                        "s h (a f) -> s h a f", f=(SR * HL * QB) // 128
                    ),
                )
                rec_sb = small.tile([128, (SR * HL * QB) // 128], F16, tag="recs")
                with nc.allow_low_precision(reason="softmax normalizer in fp16"):
                    nc.vector.reciprocal(out=rec_sb[:], in_=sums_sb[:])
                nc.sync.dma_start(
                    out=recD[qb].rearrange(
                        "s (a f) -> (s a) f", f=(SR * HL * QB) // 128
                    ),
                    in_=rec_sb[:],
                )
                # all 8 chunks' O rows + 1/sumexp broadcasts
                ot_all = fcin.tile([128, DCH, QB], F16, tag="fcin", name="ot_all")
                rs_all = fcin.tile([128, DCH, QB], F16, tag="fcrs", name="rs_all")
                for h in range(HL):
                    nc.sync.dma_start(
                        out=ot_all[h * DV : (h + 1) * DV],
                        in_=ago[:, h, :DV, :].rearrange("s x q -> x s q"),
                    )
                    nc.gpsimd.dma_start(
                        out=rs_all[h * DV : (h + 1) * DV],
                        in_=recD[qb]
                        .rearrange("(c h) q -> h c q", h=HL)[h][None, :, :]
                        .to_broadcast([DV, DCH, QB]),
                    )
                nc.vector.tensor_mul(out=ot_all[:], in0=ot_all[:], in1=rs_all[:])
                for t in range(2):  # two 128-column output tiles
                    fps = pp_fc.tile([128, QB], F32, tag="fcpsum", name="fps")
                    gps = pp_fc.tile([128, QB], F32, tag="fcpsum", name="gps")
                    for c in range(DCH):
                        nc.tensor.matmul(
                            fps[:],
                            lhsT=wfc_sb[:, c, t * 128 : (t + 1) * 128],
                            rhs=ot_all[:, c, :],
                            start=(c == 0),
                            stop=(c == DCH - 1),
                        )
                    for c in range(DCH):
                        nc.tensor.matmul(
                            gps[:],
                            lhsT=wg_sb[:, c, t * 128 : (t + 1) * 128],
                            rhs=ot_all[:, c, :],
                            start=(c == 0),
                            stop=(c == DCH - 1),
                        )
                    # sigmoid(g) = 0.5*tanh(g/2) + 0.5 — stays on the
                    # exp/tanh table set (no ~2.7us table reloads)
                    tanh_t = small.tile([128, QB], F32, tag="tanh")
                    sig_t = small.tile([128, QB], F32, tag="sig")
                    nc.scalar.activation(
                        out=tanh_t[:], in_=fps[:],
                        func=mybir.ActivationFunctionType.Tanh,
                    )
                    nc.scalar.activation(
                        out=sig_t[:], in_=gps[:],
                        func=mybir.ActivationFunctionType.Tanh, scale=0.5,
                    )
                    nc.vector.tensor_scalar(
                        out=sig_t[:],
                        in0=sig_t[:],
                        scalar1=0.5,
                        scalar2=0.5,
                        op0=mybir.AluOpType.mult,
                        op1=mybir.AluOpType.add,
                    )
                    res = small.tile([128, QB], F32, tag="res")
                    nc.vector.tensor_mul(out=res[:], in0=sig_t[:], in1=tanh_t[:])
                    nc.gpsimd.dma_start(
                        out=out[t * 128 : (t + 1) * 128, bass.ts(qb, QB)],
                        in_=res[:],
                    )

            # ---- emission order: keys/values first so attention starts
            # early; q-blocks projected just-in-time; AllGather fires per
            # q-block after both head-pairs; fc blocks slot into the last
            # attention stretches with >=1 unit of AllGather slack ----
            proj_kq(kT, wk_sb, [khTs[0][:, bass.ts(0, 512)],
                                khTs[1][:, bass.ts(0, 512)]], 0)
            # remaining weights deferred behind the first key block so the
            # startup HBM burst stays small (less cross-core stagger)
            nc.sync.dma_start(out=wv_sb[:], in_=wvT[:])
            for nt in range(1, NQB):  # keys, both head-pairs
                proj_kq(kT, wk_sb, [khTs[0][:, bass.ts(nt, 512)],
                                    khTs[1][:, bass.ts(nt, 512)]], nt)
            nc.sync.dma_start(out=wq_sb[:], in_=wqT[:])
            for nt in range(NQB):  # values
                proj_v(nt)
            nc.sync.dma_start(out=wfc_sb[:], in_=wfcT[:])
            nc.sync.dma_start(out=wg_sb[:], in_=wgT[:])
            proj_kq(qT, wq_sb, [qhTs[0], qhTs[1]], 0)
            attention(0, 0)
            proj_kq(qT, wq_sb, [qhTs[2], qhTs[3]], 1)
            attention(0, 1)
            allgather(0)
            attention(1, 0)
            proj_kq(qT, wq_sb, [qhTs[4], qhTs[5]], 2)
            attention(1, 1)
            allgather(1)
            attention(2, 0)
            proj_kq(qT, wq_sb, [qhTs[6], qhTs[7]], 3)
            attention(2, 1)
            allgather(2)
            attention(3, 0)
            fc_block(0)
            attention(3, 1)
            allgather(3)
            fc_block(1)
            fc_block(2)
            fc_block(3)

    return nc


_NC_CACHE = None


def _get_nc():
    global _NC_CACHE
    if _NC_CACHE is None:
        _NC_CACHE = build_kernel()
    return _NC_CACHE


def prepare_inputs(q, k, v, Wq, bq, Wk, bk, Wv, bv, Wfc, bfc, Wg, bg):
    """Host-side layout prep: transpose + fp16 cast + per-core slices.

    Core c = (batch c//4, head-group c%4). Activations are pre-tiled to
    [128, nt, c, n] so each DMA partition line is one contiguous 8KB run.
    Biases are structurally zero in this problem and are folded out.
    """

    def tile_act(xb):
        # [L, D] -> [D, L] -> [c, p, nt, n] -> [p, nt, c, n]
        xT = np.ascontiguousarray(np.asarray(xb).reshape(L, D).T, np.float16)
        return np.ascontiguousarray(
            xT.reshape(DCH, 128, NQB, 512).transpose(1, 2, 0, 3)
        )

    def tile_w(wT):
        # [D, M] -> [c, p, M] -> [p, c, M]
        return np.ascontiguousarray(
            wT.reshape(DCH, 128, wT.shape[1]).transpose(1, 0, 2)
        )

    acts = [[tile_act(x[b]) for b in range(B)] for x in (q, k, v)]
    WqT = (np.asarray(Wq, np.float32) / TEMP).T.astype(np.float16)  # [D, H*DK]
    WkT = np.asarray(Wk, np.float32).T.astype(np.float16)
    WvT = np.asarray(Wv, np.float32).T.astype(np.float16)
    WfcT = np.asarray(Wfc, np.float32).T.astype(np.float16)  # [H*DV, D]
    WgT = np.asarray(Wg, np.float32).T.astype(np.float16)

    in_maps = []
    for c in range(NC):
        b, g = c // NG, c % NG
        hs = g * HC * DK  # 256-wide head slice
        in_maps.append(
            {
                "qT": acts[0][b],
                "kT": acts[1][b],
                "vT": acts[2][b],
                "wqT": tile_w(WqT[:, hs : hs + HC * DK]),
                "wkT": tile_w(WkT[:, hs : hs + HC * DK]),
                "wvT": tile_w(WvT[:, hs : hs + HC * DV]),
                "wfcT": tile_w(WfcT[:, g * 256 : (g + 1) * 256]),
                "wgT": tile_w(WgT[:, g * 256 : (g + 1) * 256]),
            }
        )
    return in_maps


def assemble_output(results):
    # core (b, g) produced output columns [g*256, (g+1)*256) of batch b,
    # transposed [256, L]
    batches = []
    for b in range(B):
        cols = [results[b * NG + g]["out"] for g in range(NG)]
        full = np.concatenate(cols, axis=0)  # [D, L]
        batches.append(np.ascontiguousarray(full.T))  # [L, D]
    return np.stack(batches, axis=0)


def kernel(**inputs):
    nc = _get_nc()
    in_maps = prepare_inputs(**{k: np.asarray(v) for k, v in inputs.items()})
    res = bass_utils.run_bass_kernel_spmd(nc, in_maps, core_ids=list(range(NC)))
    return assemble_output(res.results)


if __name__ == "__main__":
    nc = build_kernel()
    print("kernel built OK")


# revision 19
# speedup vs baseline: 1.3792x; 1.0332x over previous
"""Trainium2 Bass kernel for nn_MultiHeadAttention_78134045049371.

Strategy (8 NeuronCores, batch x head hybrid sharding):
  - Cores are split into 2 groups of 4 (group = batch). Core (b, g) owns
    batch b and heads [4g, 4g+4) — together a group covers all 16 heads
    of its batch, so the fc AllGather only spans 4 ranks and the two
    groups' collectives run in parallel on disjoint dies. Per-core gather
    wire traffic drops ~2.4x vs head-only sharding, and input DMA halves
    (each core loads only its batch).
  - Host feeds per-batch q/k/v TRANSPOSED and pre-tiled ([128, nt, c, 512]
    fp16) plus per-core weight slices, so every matmul contracts over the
    partition axis and every DMA partition line is one contiguous 8KB run.
  - The 4 heads per core are processed as 2 head-pairs (hp); each
    (q-block, hp) attention unit is the classic transposed-scores loop:
    S^T [keys, q] with the two heads in PE row groups (0,0)/(64,0), exp on
    [128, 2*QB] PSUM spans, and the "ones column" trick so O_aug =
    [V|1]^T @ exp(S^T) accumulates unnormalized output + exp-sum rows.
  - Contributions ([130, QB] per hp: 2 heads x (64 O rows + sumexp)) are
    AllGathered per q-block across the 4 group ranks -> [8*130, QB]
    "super-rank" (rank, hp) layout, identical to an 8-rank head-split
    gather, so fc-side softmax normalization (wide reciprocal + DRAM
    broadcast bounce) applies unchanged.
  - Final gated projection: core (b, g) computes its 256 output columns
    (2 out-tiles of 128) of sigmoid(O@Wg^T) * tanh(O@Wfc^T) for its
    batch's 2048 rows. Output stays transposed; host reassembles.
  - Queue assignment: input streams + fc input DMAs on sync, contribution
    and result DMAs + collective triggers on gpsimd, broadcast reads on
    gpsimd. All matmuls fp16 (fp32 PSUM accumulation); exp/tanh fp32 ACT.
"""

import sys

for _p in ("/opt/trn_rl_repo", "/root/.axon_site/_ro/trn_rl_repo"):
    if _p not in sys.path:
        sys.path.append(_p)

import numpy as np

import concourse.bass as bass
import concourse.mybir as mybir
import concourse.tile as tile
from concourse import bass_utils
from concourse.vector_clock import ScopedClock

# Problem shape (fixed by the reference)
B, L, D = 2, 2048, 1024
H, DK, DV = 16, 64, 64
NC = 8  # cores
NG = 4  # cores per group (= ranks per AllGather)
HC = H // NG  # heads per core = 4
HL = 2  # heads per head-pair (attention unit)
NHP = HC // HL  # head-pairs per core = 2
TEMP = float(np.sqrt(DK))  # 8.0

NQB = 4  # q-blocks per core (one batch: L rows)
QB = L // NQB  # 512 columns per q-block
KT = 128  # key tile (partition dim of S^T)
NKT = L // KT  # 16 key tiles
DCH = D // 128  # 8 contraction chunks of 128
SR = NG * NHP  # "super-ranks" in the gathered buffer = 8
CR = HL * (DV + 1)  # contribution rows per head-pair = 130

F16 = mybir.dt.float16
F32 = mybir.dt.float32

MAX_WAITS = 1  # this walrus build encodes at most 1 sem-wait per instruction


def _split_excess_waits(nc):
    """Move excess sem-waits onto NOPs inserted just before the owning
    instruction on the same engine (engine queues are FIFO, so semantics
    are preserved). The walrus build here rejects >1 wait per instruction."""
    for f in nc.m.functions:
        for bb in f.blocks:
            out = []
            changed = False
            for inst in bb.instructions:
                si = inst.sync_info
                waits = list(si.on_wait) if si and si.on_wait else []
                if len(waits) > MAX_WAITS:
                    changed = True
                    k = 0
                    while len(waits) > MAX_WAITS:
                        chunk, waits = waits[:MAX_WAITS], waits[MAX_WAITS:]
                        nop = mybir.InstNoOp(
                            name=f"{inst.name}-wsplit-{k}", ins=[], outs=[]
                        )
                        nop.engine = inst.engine
                        nop.sync_info = mybir.SyncInfo(on_wait=chunk, on_update=[])
                        nc.register_instruction(nop, overwrite=True)
                        out.append(nop)
                        k += 1
                    si.on_wait = waits
                    inst.sync_info = si
                out.append(inst)
            if changed:
                bb.instructions = out


class _TileContext(tile.TileContext):
    """TileContext whose final drain carries its waits on separate NOPs."""

    def _drain_and_barrier(self, tick_clock, wait_clock):
        nc = self.nc
        collector = nc.sync.nop(nofuse=True)
        wait_clock.add_sem_waits(
            collector.ins, ScopedClock({None: tick_clock.global_clock})
        )
        nc.sync.drain()
        nc.all_engine_barrier()
        popped = nc._tile_sem_poison_stack.pop()
        assert popped is self._sem_poison
        nc.clear_and_free_semaphores(list(self.sems.allocated().values()))
        nc.all_engine_barrier()

    def __exit__(self, exc_type, exc_value, traceback):
        super().__exit__(exc_type, exc_value, traceback)
        if exc_type is None:
            _split_excess_waits(self.nc)


def build_kernel():
    nc = bass.Bass(target_bir_lowering=False)

    # Inputs (per core): pre-tiled transposed activations for THIS CORE'S
    # BATCH and per-core weight slices, all fp16. Layout [128, nt, c, n]:
    # partition p holds contraction-row c*128+p, column block nt.
    qT = nc.dram_tensor("qT", [128, NQB, DCH, 512], F16, kind="ExternalInput")
    kT = nc.dram_tensor("kT", [128, NQB, DCH, 512], F16, kind="ExternalInput")
    vT = nc.dram_tensor("vT", [128, NQB, DCH, 512], F16, kind="ExternalInput")
    # [128, c, 256]: columns = this core's 4 heads x 64 (q pre-scaled 1/8)
    wqT = nc.dram_tensor("wqT", [128, DCH, HC * DK], F16, kind="ExternalInput")
    wkT = nc.dram_tensor("wkT", [128, DCH, HC * DK], F16, kind="ExternalInput")
    wvT = nc.dram_tensor("wvT", [128, DCH, HC * DV], F16, kind="ExternalInput")
    # [128, c, 256]: this core's 256 output columns of Wfc/Wg, contraction
    # row c*128+p (= super-rank c's head block of H*DV)
    wfcT = nc.dram_tensor("wfcT", [128, DCH, 2 * 128], F16, kind="ExternalInput")
    wgT = nc.dram_tensor("wgT", [128, DCH, 2 * 128], F16, kind="ExternalInput")

    # Output: this core's 256 output columns for its batch's L rows,
    # stored transposed ([dout, row]); the host does the final transpose.
    out = nc.dram_tensor("out", [2 * 128, L], F32, kind="ExternalOutput")

    # AllGather buffers: per q-block contribution [2*130, QB] (per head:
    # 64 unnormalized O^T rows + exp-sum row, x2 head-pairs) -> gathered
    # [NG*260, QB] = [SR*130, QB] with super-rank (rank, hp) on dim 0.
    ag_in = nc.dram_tensor("ag_in", [NQB, NHP * CR, QB], F16)
    ag_out = nc.dram_tensor("ag_out", [NQB, NHP, NG * CR, QB], F16)
    # 1/sumexp rows, [16, QB] per q-block (SR x HL heads), bounced via
    # DRAM so they can be broadcast-read across partitions (SBUF sources
    # cannot have partition-step-0 APs, DRAM sources can)
    recD = nc.dram_tensor("recD", [NQB, SR * HL, QB], F16)

    GROUPS = [[0, 1, 2, 3], [4, 5, 6, 7]]

    with _TileContext(nc) as tc:
        with (
            tc.tile_pool(name="persist", bufs=1) as persist,
            tc.tile_pool(name="astream", bufs=4) as astream,
            tc.tile_pool(name="exps", bufs=10) as exps,
            tc.tile_pool(name="small", bufs=3) as small,
            tc.tile_pool(name="fcin", bufs=2) as fcin,
            tc.tile_pool(name="pp_o", bufs=2, space="PSUM") as pp_o,
            tc.tile_pool(name="pp_fc", bufs=2, space="PSUM") as pp_fc,
            tc.tile_pool(name="pp_s", bufs=2, space="PSUM") as pp_s,
        ):
            # ---- resident tiles (indexed by head-pair hp) ----
            qhTs = [
                persist.tile([HL * DK, QB], F16, name=f"qhT{i}")
                for i in range(NQB * NHP)  # index qb*NHP + hp
            ]
            khTs = [
                persist.tile([HL * DK, L], F16, name=f"khT{i}") for i in range(NHP)
            ]
            # vh augmented with a ones column per head: [head][0:64]=vh, [64]=1
            vhs = [
                persist.tile([128, L // 128, HL * (DV + 1)], F16, name=f"vh{i}")
                for i in range(NHP)
            ]
            wfc_sb = persist.tile([128, DCH, 2 * 128], F16)
            wg_sb = persist.tile([128, DCH, 2 * 128], F16)

            # ones columns of vh (written once; matmul copies never touch them)
            for vh in vhs:
                nc.vector.memset(vh[:, :, DV : DV + 1], 1.0)
                nc.vector.memset(vh[:, :, DV + 1 + DV :], 1.0)

            # ---- projection weights ----
            wq_sb = persist.tile([128, DCH, HC * DK], F16)
            wk_sb = persist.tile([128, DCH, HC * DK], F16)
            wv_sb = persist.tile([128, DCH, HC * DV], F16)
            nc.sync.dma_start(out=wk_sb[:], in_=wkT[:])

            # ---- projections ----
            def proj_kq(src, wsb, dsts, nt):
                # dsts[hp] [128, 512] = sum_c w[c, hp].T @ xT[c] for block nt
                xt = astream.tile([128, DCH, 512], F16, tag="xproj", name="xt")
                nc.sync.dma_start(out=xt[:], in_=src[:, nt])
                for hp in range(NHP):
                    ps = pp_fc.tile([128, 512], F32, tag="fcpsum", name="psq")
                    for c in range(DCH):
                        nc.tensor.matmul(
                            ps[:],
                            lhsT=wsb[:, c, hp * 128 : (hp + 1) * 128],
                            rhs=xt[:, c, :],
                            start=(c == 0),
                            stop=(c == DCH - 1),
                        )
                    nc.vector.tensor_copy(out=dsts[hp][:], in_=ps[:])

            def proj_v(nt):
                vt = astream.tile([128, DCH, 512], F16, tag="vproj", name="vt")
                nc.sync.dma_start(out=vt[:], in_=vT[:, nt])
                for sub in range(4):
                    loc = nt * 4 + sub
                    ps = pp_fc.tile([128, 512], F32, tag="fcpsum", name="psv")
                    for hp in range(NHP):
                        for c in range(DCH):
                            nc.tensor.matmul(
                                ps[:, hp * 128 : (hp + 1) * 128],
                                lhsT=vt[:, c, bass.ts(sub, 128)],
                                rhs=wv_sb[:, c, hp * 128 : (hp + 1) * 128],
                                start=(c == 0),
                                stop=(c == DCH - 1),
                            )
                    for hp in range(NHP):
                        for h in range(HL):
                            nc.vector.tensor_copy(
                                out=vhs[hp][
                                    :, loc, h * (DV + 1) : h * (DV + 1) + DV
                                ],
                                in_=ps[:, (hp * HL + h) * DV : (hp * HL + h + 1) * DV],
                            )

            # ---- attention per (q-block, head-pair) ----
            # S matmuls for the two heads sit at PE row groups (0,0)/(64,0)
            # and execute concurrently. exp runs on [128, 2*QB] PSUM spans.
            # Unnormalized O rows + exp-sum rows ship into the per-q-block
            # AllGather; normalization happens post-gather on the fc side.
            def attention(qb, hp):
                opsums = [
                    pp_o.tile([DV + 1, QB], F32, tag="opsum", name=f"ops{h}")
                    for h in range(HL)
                ]
                for kt in range(NKT):
                    sps = pp_s.tile([KT, HL * QB], F32, tag="spsum")
                    for h in range(HL):
                        hp_ = h * DK
                        nc.tensor.matmul(
                            sps[:, h * QB : (h + 1) * QB],
                            lhsT=khTs[hp][hp_ : hp_ + DK, kt * KT : (kt + 1) * KT],
                            rhs=qhTs[qb * NHP + hp][hp_ : hp_ + DK, :],
                            start=True,
                            stop=True,
                        )
                    et = exps.tile([KT, HL * QB], F16, tag="expst")
                    nc.scalar.activation(
                        out=et[:],
                        in_=sps[:],
                        func=mybir.ActivationFunctionType.Exp,
                    )
                    for h in range(HL):
                        nc.tensor.matmul(
                            opsums[h][:],
                            lhsT=vhs[hp][:, kt, h * (DV + 1) : (h + 1) * (DV + 1)],
                            rhs=et[:, h * QB : (h + 1) * QB],
                            start=(kt == 0),
                            stop=(kt == NKT - 1),
                        )
                for h in range(HL):
                    ctile = small.tile([DV + 1, QB], F16, tag="contrib", name="ct")
                    nc.vector.tensor_copy(out=ctile[:], in_=opsums[h][:])
                    nc.gpsimd.dma_start(
                        out=ag_in[
                            qb,
                            hp * CR + h * (DV + 1) : hp * CR + (h + 1) * (DV + 1),
                        ],
                        in_=ctile[:],
                    )

            def allgather(qb, hp):
                # per-head-pair gather: half the payload per collective, so
                # the (3,0) gather hides under attention(3,1) and the tail
                # collective is half-size
                nc.gpsimd.collective_compute(
                    "AllGather",
                    mybir.AluOpType.bypass,
                    replica_groups=GROUPS,
                    ins=[ag_in[qb, hp * CR : (hp + 1) * CR]],
                    outs=[ag_out[qb, hp]],
                )

            # ---- gated output projection for this core's 256 columns ----
            def fc_block(qb):
                # super-rank s = (rank, hp) carries heads 2s..2s+1; gathered
                # block s = contraction chunk s of Wfc/Wg
                agos = [
                    ag_out[qb, hp].rearrange("(r h x) q -> r h x q", h=HL, x=DV + 1)
                    for hp in range(NHP)
                ]
                # reciprocal of all 16 exp-sum rows at once, reshaped to
                # [128, 64] so the reciprocal runs on 128 lanes, then bounced
                # to DRAM for partition-broadcast reads.
                sums_sb = small.tile([128, (SR * HL * QB) // 128], F16, tag="sums")
                for hp in range(NHP):
                    nc.sync.dma_start(
                        out=sums_sb[hp * 64 : (hp + 1) * 64],
                        in_=agos[hp][:, :, DV, :].rearrange(
                            "r h (a f) -> r h a f", f=(SR * HL * QB) // 128
                        ),
                    )
                rec_sb = small.tile([128, (SR * HL * QB) // 128], F16, tag="recs")
                with nc.allow_low_precision(reason="softmax normalizer in fp16"):
                    nc.vector.reciprocal(out=rec_sb[:], in_=sums_sb[:])
                nc.sync.dma_start(
                    out=recD[qb].rearrange(
                        "s (a f) -> (s a) f", f=(SR * HL * QB) // 128
                    ),
                    in_=rec_sb[:],
                )
                # all 8 chunks' O rows + 1/sumexp broadcasts
                ot_all = fcin.tile([128, DCH, QB], F16, tag="fcin", name="ot_all")
                rs_all = fcin.tile([128, DCH, QB], F16, tag="fcrs", name="rs_all")
                for h in range(HL):
                    for hp in range(NHP):
                        nc.sync.dma_start(
                            out=ot_all[h * DV : (h + 1) * DV, hp * NG : (hp + 1) * NG],
                            in_=agos[hp][:, h, :DV, :].rearrange("r x q -> x r q"),
                        )
                    nc.gpsimd.dma_start(
                        out=rs_all[h * DV : (h + 1) * DV],
                        in_=recD[qb]
                        .rearrange("(c h) q -> h c q", h=HL)[h][None, :, :]
                        .to_broadcast([DV, DCH, QB]),
                    )
                nc.vector.tensor_mul(out=ot_all[:], in0=ot_all[:], in1=rs_all[:])
                for t in range(2):  # two 128-column output tiles
                    fps = pp_fc.tile([128, QB], F32, tag="fcpsum", name="fps")
                    gps = pp_fc.tile([128, QB], F32, tag="fcpsum", name="gps")
                    for c in range(DCH):
                        nc.tensor.matmul(
                            fps[:],
                            lhsT=wfc_sb[:, c, t * 128 : (t + 1) * 128],
                            rhs=ot_all[:, c, :],
                            start=(c == 0),
                            stop=(c == DCH - 1),
                        )
                    for c in range(DCH):
                        nc.tensor.matmul(
                            gps[:],
                            lhsT=wg_sb[:, c, t * 128 : (t + 1) * 128],
                            rhs=ot_all[:, c, :],
                            start=(c == 0),
                            stop=(c == DCH - 1),
                        )
                    # sigmoid(g) = 0.5*tanh(g/2) + 0.5 — stays on the
                    # exp/tanh table set (no ~2.7us table reloads)
                    tanh_t = small.tile([128, QB], F32, tag="tanh")
                    sig_t = small.tile([128, QB], F32, tag="sig")
                    nc.scalar.activation(
                        out=tanh_t[:], in_=fps[:],
                        func=mybir.ActivationFunctionType.Tanh,
                    )
                    nc.scalar.activation(
                        out=sig_t[:], in_=gps[:],
                        func=mybir.ActivationFunctionType.Tanh, scale=0.5,
                    )
                    nc.vector.tensor_scalar(
                        out=sig_t[:],
                        in0=sig_t[:],
                        scalar1=0.5,
                        scalar2=0.5,
                        op0=mybir.AluOpType.mult,
                        op1=mybir.AluOpType.add,
                    )
                    res = small.tile([128, QB], F32, tag="res")
                    nc.vector.tensor_mul(out=res[:], in0=sig_t[:], in1=tanh_t[:])
                    nc.gpsimd.dma_start(
                        out=out[t * 128 : (t + 1) * 128, bass.ts(qb, QB)],
                        in_=res[:],
                    )

            # ---- emission order: keys/values first so attention starts
            # early; q-blocks projected just-in-time; AllGather fires per
            # q-block after both head-pairs; fc blocks slot into the last
            # attention stretches with >=1 unit of AllGather slack ----
            proj_kq(kT, wk_sb, [khTs[0][:, bass.ts(0, 512)],
                                khTs[1][:, bass.ts(0, 512)]], 0)
            # remaining weights deferred behind the first key block so the
            # startup HBM burst stays small (less cross-core stagger)
            nc.sync.dma_start(out=wv_sb[:], in_=wvT[:])
            for nt in range(1, NQB):  # keys, both head-pairs
                proj_kq(kT, wk_sb, [khTs[0][:, bass.ts(nt, 512)],
                                    khTs[1][:, bass.ts(nt, 512)]], nt)
            nc.sync.dma_start(out=wq_sb[:], in_=wqT[:])
            for nt in range(NQB):  # values
                proj_v(nt)
            nc.sync.dma_start(out=wfc_sb[:], in_=wfcT[:])
            nc.sync.dma_start(out=wg_sb[:], in_=wgT[:])
            proj_kq(qT, wq_sb, [qhTs[0], qhTs[1]], 0)
            attention(0, 0)
            allgather(0, 0)
            proj_kq(qT, wq_sb, [qhTs[2], qhTs[3]], 1)
            attention(0, 1)
            allgather(0, 1)
            attention(1, 0)
            allgather(1, 0)
            proj_kq(qT, wq_sb, [qhTs[4], qhTs[5]], 2)
            attention(1, 1)
            allgather(1, 1)
            attention(2, 0)
            allgather(2, 0)
            proj_kq(qT, wq_sb, [qhTs[6], qhTs[7]], 3)
            attention(2, 1)
            allgather(2, 1)
            attention(3, 0)
            allgather(3, 0)
            fc_block(0)
            attention(3, 1)
            allgather(3, 1)
            fc_block(1)
            fc_block(2)
            fc_block(3)

    return nc


_NC_CACHE = None


def _get_nc():
    global _NC_CACHE
    if _NC_CACHE is None:
        _NC_CACHE = build_kernel()
    return _NC_CACHE


def prepare_inputs(q, k, v, Wq, bq, Wk, bk, Wv, bv, Wfc, bfc, Wg, bg):
    """Host-side layout prep: transpose + fp16 cast + per-core slices.

    Core c = (batch c//4, head-group c%4). Activations are pre-tiled to
    [128, nt, c, n] so each DMA partition line is one contiguous 8KB run.
    Biases are structurally zero in this problem and are folded out.
    """

    def tile_act(xb):
        # [L, D] -> [D, L] -> [c, p, nt, n] -> [p, nt, c, n]
        xT = np.ascontiguousarray(np.asarray(xb).reshape(L, D).T, np.float16)
        return np.ascontiguousarray(
            xT.reshape(DCH, 128, NQB, 512).transpose(1, 2, 0, 3)
        )

    def tile_w(wT):
        # [D, M] -> [c, p, M] -> [p, c, M]
        return np.ascontiguousarray(
            wT.reshape(DCH, 128, wT.shape[1]).transpose(1, 0, 2)
        )

    acts = [[tile_act(x[b]) for b in range(B)] for x in (q, k, v)]
    WqT = (np.asarray(Wq, np.float32) / TEMP).T.astype(np.float16)  # [D, H*DK]
    WkT = np.asarray(Wk, np.float32).T.astype(np.float16)
    WvT = np.asarray(Wv, np.float32).T.astype(np.float16)
    WfcT = np.asarray(Wfc, np.float32).T.astype(np.float16)  # [H*DV, D]
    WgT = np.asarray(Wg, np.float32).T.astype(np.float16)
    # gathered layout is hp-major: super-rank s = hp*NG + r carries heads
    # 4*(s%NG) + 2*(s//NG) + {0,1}; permute the fc contraction rows to match
    perm = np.concatenate(
        [
            np.arange(DV) + (4 * (s % NG) + 2 * (s // NG) + h) * DV
            for s in range(SR)
            for h in range(HL)
        ]
    )
    WfcT = WfcT[perm]
    WgT = WgT[perm]

    in_maps = []
    for c in range(NC):
        b, g = c // NG, c % NG
        hs = g * HC * DK  # 256-wide head slice
        in_maps.append(
            {
                "qT": acts[0][b],
                "kT": acts[1][b],
                "vT": acts[2][b],
                "wqT": tile_w(WqT[:, hs : hs + HC * DK]),
                "wkT": tile_w(WkT[:, hs : hs + HC * DK]),
                "wvT": tile_w(WvT[:, hs : hs + HC * DV]),
                "wfcT": tile_w(WfcT[:, g * 256 : (g + 1) * 256]),
                "wgT": tile_w(WgT[:, g * 256 : (g + 1) * 256]),
            }
        )
    return in_maps


def assemble_output(results):
    # core (b, g) produced output columns [g*256, (g+1)*256) of batch b,
    # transposed [256, L]
    batches = []
    for b in range(B):
        cols = [results[b * NG + g]["out"] for g in range(NG)]
        full = np.concatenate(cols, axis=0)  # [D, L]
        batches.append(np.ascontiguousarray(full.T))  # [L, D]
    return np.stack(batches, axis=0)


def kernel(**inputs):
    nc = _get_nc()
    in_maps = prepare_inputs(**{k: np.asarray(v) for k, v in inputs.items()})
    res = bass_utils.run_bass_kernel_spmd(nc, in_maps, core_ids=list(range(NC)))
    return assemble_output(res.results)


if __name__ == "__main__":
    nc = build_kernel()
    print("kernel built OK")
